# revision 2
# baseline (speedup 1.0000x reference)
"""Trainium2 Bass kernel for batched 8x8-block 2D DCT — v2.

Input  x: (32, 3, 512, 512) f32, dct_basis: (8, 8) f32.
Output y: (32, 3, 512, 512) f32 with each 8x8 block B replaced by D @ B @ D^T.

Sharding: data-parallel over batch — 32 batches -> 8 NeuronCores x 4.
Each core runs an identical (SPMD) Bass program over its (4,3,512,512)
slice viewed as [6144, 512] = 24 supertiles of [128 partitions, 1024 free]
(2 row-bands x 512 cols, 512 KiB each).

Per supertile (all engines overlapped across supertiles):
    MM1: pt = Bblk @ X          PE matmul, f32r (1 cyc/row vs 4 for f32)
    tr1: t1f = blktrans32(pt)   DVE stream transpose, PSUM -> SBUF, f32
    rnd: t1 = f32r(t1f)         ACT copy (BIR requires f32r-rounded input)
    MM2: py = Bblk @ t1         PE matmul, f32r
    tr2: ys = blktrans32(py)    DVE stream transpose, PSUM -> SBUF, f32
    out: y[s] = bf16(ys)        gpsimd (SWDGE) DMA casts f32->bf16 in-flight
where Bblk = kron(I_16, D) is block-diagonal [128,128]; because 8 | 32, the
32x32 block transpose commutes with the per-8-block DCT on both axes.

HBM traffic per core: 12 MiB f32 in + 6 MiB bf16 out (the output is stored
bf16 and upcast host-side; global rel err ~3e-3, dominated by the bf16
quantization, vs the 2e-2 tolerance). f32r matmul precision costs ~1e-4.
Input DMAs ride the SP HWDGE ring; output DMAs ride the gpsimd SWDGE ring
(which also performs the dtype cast); ACT does one rounding copy; PE/DVE/
ACT/SP/Pool all stay below the ~2.4 us/supertile DMA beat.
"""

import sys

for _p in ("/opt/trn_rl_repo",):
    if _p not in sys.path:
        sys.path.insert(0, _p)

from contextlib import ExitStack

import numpy as np

N_CORES = 8
B, C, H, W = 32, 3, 512, 512
ROWS_PER_CORE = (B // N_CORES) * C * H  # 6144
N_SUPER = ROWS_PER_CORE // 256  # 24

# mode: "v4r" (f32r both stages + ACT rounding copy, bf16 cast-DMA out)
#       "v4c" (f32r stage1, f32 stage2, bf16 cast-DMA out)
#       "v4r_f32out" (f32r both stages, f32 out)
MODE = "v4r"

_NC_CACHE = {}


def _build_nc(mode=MODE, rep=1, hwloop=0, bufs=4):
    import concourse.bacc as bacc
    import concourse.tile as tile
    import concourse.mybir as mybir

    F32 = mybir.dt.float32
    F32R = mybir.dt.float32r
    BF16 = mybir.dt.bfloat16

    stage2 = "f32" if mode == "v4c" else "f32r_round"
    bf16_out = mode != "v4r_f32out"
    YDT = BF16 if bf16_out else F32

    nc = bacc.Bacc("TRN2", target_bir_lowering=False, debug=False,
                   enable_asserts=False)
    x_ap = nc.dram_tensor("x", [ROWS_PER_CORE, 512], F32R,
                          kind="ExternalInput").ap()
    bt_ap = nc.dram_tensor("bt", [128, 128], F32R, kind="ExternalInput").ap()
    if stage2 == "f32":
        bt2_ap = nc.dram_tensor("bt2", [128, 128], F32,
                                kind="ExternalInput").ap()
    y_ap = nc.dram_tensor("y", [ROWS_PER_CORE, 512], YDT,
                          kind="ExternalOutput").ap()

    with tile.TileContext(nc) as tc, ExitStack() as ctx:
        xv = x_ap.rearrange("(n t p) w -> n p t w", t=2, p=128)
        yv = y_ap.rearrange("(n t p) w -> n p t w", t=2, p=128)

        def as3d(sb_ap):
            return sb_ap.rearrange("p (t w) -> p t w", t=2)

        const = ctx.enter_context(tc.tile_pool(name="const", bufs=1))
        bt = const.tile([128, 128], F32R)
        nc.gpsimd.dma_start(bt[:], bt_ap)
        if stage2 == "f32":
            bt2 = const.tile([128, 128], F32)
            nc.gpsimd.dma_start(bt2[:], bt2_ap)
        else:
            bt2 = bt

        xp = ctx.enter_context(tc.tile_pool(name="xp", bufs=bufs))
        tp = ctx.enter_context(tc.tile_pool(name="tp", bufs=bufs))
        yp = ctx.enter_context(tc.tile_pool(name="yp", bufs=bufs))
        cpp = ctx.enter_context(tc.tile_pool(name="cpp", bufs=bufs))
        pst = ctx.enter_context(tc.tile_pool(name="pst", bufs=2, space="PSUM"))
        psy = ctx.enter_context(tc.tile_pool(name="psy", bufs=2, space="PSUM"))

        loop_ctx = tc.For_i(0, hwloop) if hwloop else None
        if loop_ctx is not None:
            ctx.enter_context(loop_ctx)
        for _ in range(rep):
            for s in range(N_SUPER):
                xs = xp.tile([128, 1024], F32R)
                nc.sync.dma_start(as3d(xs[:]), xv[s])

                pt = pst.tile([128, 1024], F32)
                for h in range(2):
                    nc.tensor.matmul(pt[:, h * 512:(h + 1) * 512], bt[:],
                                     xs[:, h * 512:(h + 1) * 512],
                                     start=True, stop=True)

                if stage2 == "f32r_round":
                    t1f = tp.tile([128, 1024], F32)
                    nc.vector.transpose(t1f[:], pt[:])
                    t1 = cpp.tile([128, 1024], F32R)
                    nc.scalar.copy(t1[:], t1f[:])
                else:
                    t1 = tp.tile([128, 1024], F32)
                    nc.vector.transpose(t1[:], pt[:])

                py = psy.tile([128, 1024], F32)
                for h in range(2):
                    nc.tensor.matmul(py[:, h * 512:(h + 1) * 512], bt2[:],
                                     t1[:, h * 512:(h + 1) * 512],
                                     start=True, stop=True)

                ys = yp.tile([128, 1024], F32)
                nc.vector.transpose(ys[:], py[:])
                if bf16_out:
                    nc.gpsimd.dma_start(yv[s], as3d(ys[:]))  # SWDGE f32->bf16
                else:
                    nc.gpsimd.dma_start(yv[s], as3d(ys[:]))

    nc.compile()
    return nc


def _get_nc(mode=MODE, rep=1, hwloop=0):
    key = (mode, rep, hwloop)
    if key not in _NC_CACHE:
        _NC_CACHE[key] = _build_nc(mode=mode, rep=rep, hwloop=hwloop)
    return _NC_CACHE[key]


def run_sharded(x, dct_basis, mode=MODE):
    """Shard batch over 8 cores, run the Bass kernel SPMD, gather output."""
    from concourse import bass_utils

    x = np.ascontiguousarray(np.asarray(x), dtype=np.float32)
    dct_basis = np.asarray(dct_basis, dtype=np.float32)
    assert x.shape == (B, C, H, W), x.shape

    bt = np.ascontiguousarray(
        np.kron(np.eye(16, dtype=np.float32), dct_basis).T.astype(np.float32)
    )
    bpc = B // N_CORES
    in_maps = []
    for c in range(N_CORES):
        m = {"x": x[c * bpc:(c + 1) * bpc].reshape(ROWS_PER_CORE, 512),
             "bt": bt}
        if mode == "v4c":
            m["bt2"] = bt
        in_maps.append(m)
    nc = _get_nc(mode=mode)
    res = bass_utils.run_bass_kernel_spmd(nc, in_maps, list(range(N_CORES)))
    out = np.concatenate(
        [np.asarray(res.results[c]["y"]).astype(np.float32).reshape(
            bpc, C, H, W) for c in range(N_CORES)],
        axis=0,
    )
    return out


def kernel(x, dct_basis):
    return run_sharded(x, dct_basis, mode=MODE)


# revision 7
# speedup vs baseline: 1.0430x; 1.0430x over previous
"""Trainium2 Bass kernel for batched 8x8-block 2D DCT — v2.

Input  x: (32, 3, 512, 512) f32, dct_basis: (8, 8) f32.
Output y: (32, 3, 512, 512) f32 with each 8x8 block B replaced by D @ B @ D^T.

Sharding: data-parallel over batch — 32 batches -> 8 NeuronCores x 4.
Each core runs an identical (SPMD) Bass program over its (4,3,512,512)
slice viewed as [6144, 512] = 24 supertiles of [128 partitions, 1024 free]
(2 row-bands x 512 cols, 512 KiB each).

Per supertile (all engines overlapped across supertiles):
    MM1: pt = Bblk @ X          PE matmul, f32r (1 cyc/row vs 4 for f32)
    tr1: t1f = blktrans32(pt)   DVE stream transpose, PSUM -> SBUF, f32
    rnd: t1 = f32r(t1f)         ACT copy (BIR requires f32r-rounded input)
    MM2: py = Bblk @ t1         PE matmul, f32r
    tr2: ys = blktrans32(py)    DVE stream transpose, PSUM -> SBUF, f32
    out: y[s] = bf16(ys)        gpsimd (SWDGE) DMA casts f32->bf16 in-flight
where Bblk = kron(I_16, D) is block-diagonal [128,128]; because 8 | 32, the
32x32 block transpose commutes with the per-8-block DCT on both axes.

HBM traffic per core: 12 MiB f32 in + 6 MiB bf16 out (the output is stored
bf16 and upcast host-side; global rel err ~3e-3, dominated by the bf16
quantization, vs the 2e-2 tolerance). f32r matmul precision costs ~1e-4.
Input DMAs ride the SP HWDGE ring; output DMAs ride the gpsimd SWDGE ring
(which also performs the dtype cast); ACT does one rounding copy; PE/DVE/
ACT/SP/Pool all stay below the ~2.4 us/supertile DMA beat.
"""

import sys

for _p in ("/opt/trn_rl_repo",):
    if _p not in sys.path:
        sys.path.insert(0, _p)

from contextlib import ExitStack

import numpy as np

N_CORES = 8
B, C, H, W = 32, 3, 512, 512
ROWS_PER_CORE = (B // N_CORES) * C * H  # 6144
N_SUPER = ROWS_PER_CORE // 256  # 24

# mode: "v4r" (f32r both stages + ACT rounding copy, bf16 cast-DMA out;
#              512 KiB in-DMAs on SP ring)
#       "v4w2" (same pipeline, 1 MiB in-DMAs: [128,2048] supertiles computed
#               in halves, single 512 KiB bf16 out-DMA)
#       "v4c" (f32r stage1, f32 stage2, bf16 cast-DMA out)
#       "v4r_f32out" (f32r both stages, f32 out)
MODE = "v4r"

_NC_CACHE = {}


def _build_nc(mode=MODE, rep=1, hwloop=0, bufs=4):
    import concourse.bacc as bacc
    import concourse.tile as tile
    import concourse.mybir as mybir

    F32 = mybir.dt.float32
    F32R = mybir.dt.float32r
    BF16 = mybir.dt.bfloat16

    stage2 = "f32" if mode == "v4c" else "f32r_round"
    bf16_out = mode != "v4r_f32out"
    YDT = BF16 if bf16_out else F32

    nc = bacc.Bacc("TRN2", target_bir_lowering=False, debug=False,
                   enable_asserts=False)
    x_ap = nc.dram_tensor("x", [ROWS_PER_CORE, 512], F32R,
                          kind="ExternalInput").ap()
    bt_ap = nc.dram_tensor("bt", [128, 128], F32R, kind="ExternalInput").ap()
    if stage2 == "f32":
        bt2_ap = nc.dram_tensor("bt2", [128, 128], F32,
                                kind="ExternalInput").ap()
    y_ap = nc.dram_tensor("y", [ROWS_PER_CORE, 512], YDT,
                          kind="ExternalOutput").ap()

    nbands = 4 if mode == "v4w2" else 2
    nsup = ROWS_PER_CORE // (128 * nbands)

    with tile.TileContext(nc) as tc, ExitStack() as ctx:
        xv = x_ap.rearrange("(n t p) w -> n p t w", t=nbands, p=128)
        yv = y_ap.rearrange("(n t p) w -> n p t w", t=nbands, p=128)

        def as3d(sb_ap):
            return sb_ap.rearrange("p (t w) -> p t w", t=nbands)

        const = ctx.enter_context(tc.tile_pool(name="const", bufs=1))
        bt = const.tile([128, 128], F32R)
        nc.gpsimd.dma_start(bt[:], bt_ap)
        if stage2 == "f32":
            bt2 = const.tile([128, 128], F32)
            nc.gpsimd.dma_start(bt2[:], bt2_ap)
        else:
            bt2 = bt

        xp = ctx.enter_context(tc.tile_pool(name="xp", bufs=bufs))
        tp = ctx.enter_context(tc.tile_pool(name="tp", bufs=bufs))
        yp = ctx.enter_context(tc.tile_pool(name="yp", bufs=bufs))
        cpp = ctx.enter_context(tc.tile_pool(name="cpp", bufs=bufs))
        pst = ctx.enter_context(tc.tile_pool(name="pst", bufs=2, space="PSUM"))
        psy = ctx.enter_context(tc.tile_pool(name="psy", bufs=2, space="PSUM"))

        def halves(xh, pipe_t1, pipe_ys):
            """One [128,1024] compute beat: MM1 -> tr1 -> round -> MM2 -> tr2."""
            pt = pst.tile([128, 1024], F32)
            for h in range(2):
                nc.tensor.matmul(pt[:, h * 512:(h + 1) * 512], bt[:],
                                 xh[:, h * 512:(h + 1) * 512],
                                 start=True, stop=True)
            if stage2 == "f32r_round":
                t1f = pipe_t1.tile([128, 1024], F32)
                nc.vector.transpose(t1f[:], pt[:])
                t1 = cpp.tile([128, 1024], F32R)
                nc.scalar.copy(t1[:], t1f[:])
            else:
                t1 = pipe_t1.tile([128, 1024], F32)
                nc.vector.transpose(t1[:], pt[:])
            py = psy.tile([128, 1024], F32)
            for h in range(2):
                nc.tensor.matmul(py[:, h * 512:(h + 1) * 512], bt2[:],
                                 t1[:, h * 512:(h + 1) * 512],
                                 start=True, stop=True)
            nc.vector.transpose(pipe_ys, py[:])

        loop_ctx = tc.For_i(0, hwloop) if hwloop else None
        if loop_ctx is not None:
            ctx.enter_context(loop_ctx)
        for _ in range(rep):
            for s in range(nsup):
                xs = xp.tile([128, 512 * nbands], F32R)
                nc.sync.dma_start(as3d(xs[:]), xv[s])
                ys = yp.tile([128, 512 * nbands], F32)
                for g in range(nbands // 2):
                    halves(xs[:, g * 1024:(g + 1) * 1024], tp,
                           ys[:, g * 1024:(g + 1) * 1024])
                # SWDGE out-DMA: casts f32->bf16 in-flight when bf16_out
                nc.gpsimd.dma_start(yv[s], as3d(ys[:]))

    nc.compile()
    return nc


def _get_nc(mode=MODE, rep=1, hwloop=0):
    key = (mode, rep, hwloop)
    if key not in _NC_CACHE:
        _NC_CACHE[key] = _build_nc(mode=mode, rep=rep, hwloop=hwloop)
    return _NC_CACHE[key]


def run_sharded(x, dct_basis, mode=MODE):
    """Shard batch over 8 cores, run the Bass kernel SPMD, gather output."""
    from concourse import bass_utils

    x = np.ascontiguousarray(np.asarray(x), dtype=np.float32)
    dct_basis = np.asarray(dct_basis, dtype=np.float32)
    assert x.shape == (B, C, H, W), x.shape

    bt = np.ascontiguousarray(
        np.kron(np.eye(16, dtype=np.float32), dct_basis).T.astype(np.float32)
    )
    bpc = B // N_CORES
    in_maps = []
    for c in range(N_CORES):
        m = {"x": x[c * bpc:(c + 1) * bpc].reshape(ROWS_PER_CORE, 512),
             "bt": bt}
        if mode == "v4c":
            m["bt2"] = bt
        in_maps.append(m)
    nc = _get_nc(mode=mode)
    res = bass_utils.run_bass_kernel_spmd(nc, in_maps, list(range(N_CORES)))
    out = np.concatenate(
        [np.asarray(res.results[c]["y"]).astype(np.float32).reshape(
            bpc, C, H, W) for c in range(N_CORES)],
        axis=0,
    )
    return out


def kernel(x, dct_basis):
    return run_sharded(x, dct_basis, mode=MODE)


# revision 11
# speedup vs baseline: 1.1488x; 1.1015x over previous
"""Trainium2 Bass kernel for batched 8x8-block 2D DCT — v2.

Input  x: (32, 3, 512, 512) f32, dct_basis: (8, 8) f32.
Output y: (32, 3, 512, 512) f32 with each 8x8 block B replaced by D @ B @ D^T.

Sharding: data-parallel over batch — 32 batches -> 8 NeuronCores x 4.
Each core runs an identical (SPMD) Bass program over its (4,3,512,512)
slice viewed as [6144, 512] = 24 supertiles of [128 partitions, 1024 free]
(2 row-bands x 512 cols, 512 KiB each).

Per supertile (all engines overlapped across supertiles):
    MM1: pt = Bblk @ X          PE matmul, f32r (1 cyc/row vs 4 for f32)
    tr1: t1f = blktrans32(pt)   DVE stream transpose, PSUM -> SBUF, f32
    rnd: t1 = f32r(t1f)         ACT copy (BIR requires f32r-rounded input)
    MM2: py = Bblk @ t1         PE matmul, f32r
    tr2: ys = blktrans32(py)    DVE stream transpose, PSUM -> SBUF, f32
    out: y[s] = bf16(ys)        gpsimd (SWDGE) DMA casts f32->bf16 in-flight
where Bblk = kron(I_16, D) is block-diagonal [128,128]; because 8 | 32, the
32x32 block transpose commutes with the per-8-block DCT on both axes.

HBM traffic per core: 12 MiB f32 in + 6 MiB bf16 out (the output is stored
bf16 and upcast host-side; global rel err ~3e-3, dominated by the bf16
quantization, vs the 2e-2 tolerance). f32r matmul precision costs ~1e-4.
Input DMAs ride the SP HWDGE ring; output DMAs ride the gpsimd SWDGE ring
(which also performs the dtype cast); ACT does one rounding copy; PE/DVE/
ACT/SP/Pool all stay below the ~2.4 us/supertile DMA beat.
"""

import sys

for _p in ("/opt/trn_rl_repo",):
    if _p not in sys.path:
        sys.path.insert(0, _p)

from contextlib import ExitStack

import numpy as np

N_CORES = 8
B, C, H, W = 32, 3, 512, 512
ROWS_PER_CORE = (B // N_CORES) * C * H  # 6144
N_SUPER = ROWS_PER_CORE // 256  # 24

# mode: "v4r" (f32r both stages + ACT rounding copy, bf16 cast-DMA out;
#              512 KiB in-DMAs on SP ring)
#       "v4o2" (v4r + two supertiles batched per out-DMA: 512 KiB SWDGE casts)
#       "v4w2" (same pipeline, 1 MiB in-DMAs: [128,2048] supertiles computed
#               in halves, single 512 KiB bf16 out-DMA)
#       "v4c" (f32r stage1, f32 stage2, bf16 cast-DMA out)
#       "v4r_f32out" (f32r both stages, f32 out)
MODE = "v4r"

_NC_CACHE = {}


def _build_nc(mode=MODE, rep=1, hwloop=0, bufs=6):
    import concourse.bacc as bacc
    import concourse.tile as tile
    import concourse.mybir as mybir

    F32 = mybir.dt.float32
    F32R = mybir.dt.float32r
    BF16 = mybir.dt.bfloat16

    stage2 = "f32" if mode == "v4c" else "f32r_round"
    bf16_out = mode != "v4r_f32out"
    YDT = BF16 if bf16_out else F32

    nc = bacc.Bacc("TRN2", target_bir_lowering=False, debug=False,
                   enable_asserts=False)
    x_ap = nc.dram_tensor("x", [ROWS_PER_CORE, 512], F32R,
                          kind="ExternalInput").ap()
    bt_ap = nc.dram_tensor("bt", [128, 128], F32R, kind="ExternalInput").ap()
    if stage2 == "f32":
        bt2_ap = nc.dram_tensor("bt2", [128, 128], F32,
                                kind="ExternalInput").ap()
    y_ap = nc.dram_tensor("y", [ROWS_PER_CORE, 512], YDT,
                          kind="ExternalOutput").ap()

    nbands = 4 if mode == "v4w2" else 2
    nsup = ROWS_PER_CORE // (128 * nbands)

    with tile.TileContext(nc) as tc, ExitStack() as ctx:
        xv = x_ap.rearrange("(n t p) w -> n p t w", t=nbands, p=128)
        yv = y_ap.rearrange("(n t p) w -> n p t w", t=nbands, p=128)

        def as3d(sb_ap):
            return sb_ap.rearrange("p (t w) -> p t w", t=nbands)

        const = ctx.enter_context(tc.tile_pool(name="const", bufs=1))
        bt = const.tile([128, 128], F32R)
        nc.gpsimd.dma_start(bt[:], bt_ap)
        if stage2 == "f32":
            bt2 = const.tile([128, 128], F32)
            nc.gpsimd.dma_start(bt2[:], bt2_ap)
        else:
            bt2 = bt

        xp = ctx.enter_context(tc.tile_pool(name="xp", bufs=bufs))
        tp = ctx.enter_context(tc.tile_pool(name="tp", bufs=bufs))
        yp = ctx.enter_context(tc.tile_pool(name="yp", bufs=bufs))
        cpp = ctx.enter_context(tc.tile_pool(name="cpp", bufs=bufs))
        pst = ctx.enter_context(tc.tile_pool(name="pst", bufs=2, space="PSUM"))
        psy = ctx.enter_context(tc.tile_pool(name="psy", bufs=2, space="PSUM"))

        def halves(xh, pipe_t1, pipe_ys):
            """One [128,1024] compute beat: MM1 -> tr1 -> round -> MM2 -> tr2."""
            pt = pst.tile([128, 1024], F32)
            for h in range(2):
                nc.tensor.matmul(pt[:, h * 512:(h + 1) * 512], bt[:],
                                 xh[:, h * 512:(h + 1) * 512],
                                 start=True, stop=True)
            if stage2 == "f32r_round":
                t1f = pipe_t1.tile([128, 1024], F32)
                nc.vector.transpose(t1f[:], pt[:])
                t1 = cpp.tile([128, 1024], F32R)
                nc.scalar.copy(t1[:], t1f[:])
            else:
                t1 = pipe_t1.tile([128, 1024], F32)
                nc.vector.transpose(t1[:], pt[:])
            py = psy.tile([128, 1024], F32)
            for h in range(2):
                nc.tensor.matmul(py[:, h * 512:(h + 1) * 512], bt2[:],
                                 t1[:, h * 512:(h + 1) * 512],
                                 start=True, stop=True)
            nc.vector.transpose(pipe_ys, py[:])

        out_pair = mode == "v4o2"
        yv4 = y_ap.rearrange("(n t p) w -> n p t w", t=2 * nbands, p=128)

        loop_ctx = tc.For_i(0, hwloop) if hwloop else None
        if loop_ctx is not None:
            ctx.enter_context(loop_ctx)
        ys_pair = [None]
        for _ in range(rep):
            for s in range(nsup):
                xs = xp.tile([128, 512 * nbands], F32R)
                nc.sync.dma_start(as3d(xs[:]), xv[s])
                if out_pair:
                    if s % 2 == 0:
                        ysb = yp.tile([128, 1024 * nbands], F32)
                        ys_pair[0] = ysb
                    ys = ys_pair[0][:, (s % 2) * 1024:(s % 2 + 1) * 1024]
                else:
                    ysb = yp.tile([128, 512 * nbands], F32)
                    ys = ysb[:]
                for g in range(nbands // 2):
                    halves(xs[:, g * 1024:(g + 1) * 1024], tp,
                           ys[:, g * 1024:(g + 1) * 1024])
                # SWDGE out-DMA: casts f32->bf16 in-flight when bf16_out
                if out_pair:
                    if s % 2 == 1:
                        nc.gpsimd.dma_start(
                            yv4[s // 2],
                            ys_pair[0][:].rearrange("p (t w) -> p t w",
                                                    t=2 * nbands))
                else:
                    nc.gpsimd.dma_start(yv[s], as3d(ys))

    nc.compile()
    return nc


def _get_nc(mode=MODE, rep=1, hwloop=0):
    key = (mode, rep, hwloop)
    if key not in _NC_CACHE:
        _NC_CACHE[key] = _build_nc(mode=mode, rep=rep, hwloop=hwloop)
    return _NC_CACHE[key]


def run_sharded(x, dct_basis, mode=MODE):
    """Shard batch over 8 cores, run the Bass kernel SPMD, gather output."""
    from concourse import bass_utils

    x = np.ascontiguousarray(np.asarray(x), dtype=np.float32)
    dct_basis = np.asarray(dct_basis, dtype=np.float32)
    assert x.shape == (B, C, H, W), x.shape

    bt = np.ascontiguousarray(
        np.kron(np.eye(16, dtype=np.float32), dct_basis).T.astype(np.float32)
    )
    bpc = B // N_CORES
    in_maps = []
    for c in range(N_CORES):
        m = {"x": x[c * bpc:(c + 1) * bpc].reshape(ROWS_PER_CORE, 512),
             "bt": bt}
        if mode == "v4c":
            m["bt2"] = bt
        in_maps.append(m)
    nc = _get_nc(mode=mode)
    res = bass_utils.run_bass_kernel_spmd(nc, in_maps, list(range(N_CORES)))
    out = np.concatenate(
        [np.asarray(res.results[c]["y"]).astype(np.float32).reshape(
            bpc, C, H, W) for c in range(N_CORES)],
        axis=0,
    )
    return out


def kernel(x, dct_basis):
    return run_sharded(x, dct_basis, mode=MODE)


# revision 13
# speedup vs baseline: 1.2540x; 1.0915x over previous
"""Trainium2 Bass kernel for batched 8x8-block 2D DCT — v2.

Input  x: (32, 3, 512, 512) f32, dct_basis: (8, 8) f32.
Output y: (32, 3, 512, 512) f32 with each 8x8 block B replaced by D @ B @ D^T.

Sharding: data-parallel over batch — 32 batches -> 8 NeuronCores x 4.
Each core runs an identical (SPMD) Bass program over its (4,3,512,512)
slice viewed as [6144, 512] = 24 supertiles of [128 partitions, 1024 free]
(2 row-bands x 512 cols, 512 KiB each).

Per supertile (all engines overlapped across supertiles):
    MM1: pt = Bblk @ X          PE matmul, f32r (1 cyc/row vs 4 for f32)
    tr1: t1f = blktrans32(pt)   DVE stream transpose, PSUM -> SBUF, f32
    rnd: t1 = f32r(t1f)         ACT copy (BIR requires f32r-rounded input)
    MM2: py = Bblk @ t1         PE matmul, f32r
    tr2: ys = blktrans32(py)    DVE stream transpose, PSUM -> SBUF, f32
    out: y[s] = bf16(ys)        gpsimd (SWDGE) DMA casts f32->bf16 in-flight
where Bblk = kron(I_16, D) is block-diagonal [128,128]; because 8 | 32, the
32x32 block transpose commutes with the per-8-block DCT on both axes.

HBM traffic per core: 12 MiB f32 in + 6 MiB bf16 out (the output is stored
bf16 and upcast host-side; global rel err ~3e-3, dominated by the bf16
quantization, vs the 2e-2 tolerance). f32r matmul precision costs ~1e-4.
Input DMAs ride the SP HWDGE ring; output DMAs ride the gpsimd SWDGE ring
(which also performs the dtype cast); ACT does one rounding copy; PE/DVE/
ACT/SP/Pool all stay below the ~2.4 us/supertile DMA beat.
"""

import sys

for _p in ("/opt/trn_rl_repo",):
    if _p not in sys.path:
        sys.path.insert(0, _p)

from contextlib import ExitStack

import numpy as np

N_CORES = 8
B, C, H, W = 32, 3, 512, 512
ROWS_PER_CORE = (B // N_CORES) * C * H  # 6144
N_SUPER = ROWS_PER_CORE // 256  # 24

# mode: "v4r" (f32r both stages + ACT rounding copy, bf16 cast-DMA out;
#              512 KiB in-DMAs on SP ring)
#       "v4o2" (v4r + two supertiles batched per out-DMA: 512 KiB SWDGE casts)
#       "v4w2" (same pipeline, 1 MiB in-DMAs: [128,2048] supertiles computed
#               in halves, single 512 KiB bf16 out-DMA)
#       "v4c" (f32r stage1, f32 stage2, bf16 cast-DMA out)
#       "v4r_f32out" (f32r both stages, f32 out)
MODE = "v4r"

_NC_CACHE = {}


def _build_nc(mode=MODE, rep=1, hwloop=0, bufs=8):
    import concourse.bacc as bacc
    import concourse.tile as tile
    import concourse.mybir as mybir

    F32 = mybir.dt.float32
    F32R = mybir.dt.float32r
    BF16 = mybir.dt.bfloat16

    stage2 = "f32" if mode == "v4c" else "f32r_round"
    bf16_out = mode != "v4r_f32out"
    YDT = BF16 if bf16_out else F32

    nc = bacc.Bacc("TRN2", target_bir_lowering=False, debug=False,
                   enable_asserts=False)
    x_ap = nc.dram_tensor("x", [ROWS_PER_CORE, 512], F32R,
                          kind="ExternalInput").ap()
    bt_ap = nc.dram_tensor("bt", [128, 128], F32R, kind="ExternalInput").ap()
    if stage2 == "f32":
        bt2_ap = nc.dram_tensor("bt2", [128, 128], F32,
                                kind="ExternalInput").ap()
    y_ap = nc.dram_tensor("y", [ROWS_PER_CORE, 512], YDT,
                          kind="ExternalOutput").ap()

    nbands = 4 if mode == "v4w2" else 2
    nsup = ROWS_PER_CORE // (128 * nbands)

    with tile.TileContext(nc) as tc, ExitStack() as ctx:
        xv = x_ap.rearrange("(n t p) w -> n p t w", t=nbands, p=128)
        yv = y_ap.rearrange("(n t p) w -> n p t w", t=nbands, p=128)

        def as3d(sb_ap):
            return sb_ap.rearrange("p (t w) -> p t w", t=nbands)

        const = ctx.enter_context(tc.tile_pool(name="const", bufs=1))
        bt = const.tile([128, 128], F32R)
        nc.gpsimd.dma_start(bt[:], bt_ap)
        if stage2 == "f32":
            bt2 = const.tile([128, 128], F32)
            nc.gpsimd.dma_start(bt2[:], bt2_ap)
        else:
            bt2 = bt

        xp = ctx.enter_context(tc.tile_pool(name="xp", bufs=bufs))
        tp = ctx.enter_context(tc.tile_pool(name="tp", bufs=bufs))
        yp = ctx.enter_context(tc.tile_pool(name="yp", bufs=bufs))
        cpp = ctx.enter_context(tc.tile_pool(name="cpp", bufs=bufs))
        pst = ctx.enter_context(tc.tile_pool(name="pst", bufs=2, space="PSUM"))
        psy = ctx.enter_context(tc.tile_pool(name="psy", bufs=2, space="PSUM"))

        def halves(xh, pipe_t1, pipe_ys):
            """One [128,1024] compute beat: MM1 -> tr1 -> round -> MM2 -> tr2."""
            pt = pst.tile([128, 1024], F32)
            for h in range(2):
                nc.tensor.matmul(pt[:, h * 512:(h + 1) * 512], bt[:],
                                 xh[:, h * 512:(h + 1) * 512],
                                 start=True, stop=True)
            if stage2 == "f32r_round":
                t1f = pipe_t1.tile([128, 1024], F32)
                nc.vector.transpose(t1f[:], pt[:])
                t1 = cpp.tile([128, 1024], F32R)
                nc.scalar.copy(t1[:], t1f[:])
            else:
                t1 = pipe_t1.tile([128, 1024], F32)
                nc.vector.transpose(t1[:], pt[:])
            py = psy.tile([128, 1024], F32)
            for h in range(2):
                nc.tensor.matmul(py[:, h * 512:(h + 1) * 512], bt2[:],
                                 t1[:, h * 512:(h + 1) * 512],
                                 start=True, stop=True)
            nc.vector.transpose(pipe_ys, py[:])

        out_pair = mode == "v4o2"
        yv4 = y_ap.rearrange("(n t p) w -> n p t w", t=2 * nbands, p=128)

        loop_ctx = tc.For_i(0, hwloop) if hwloop else None
        if loop_ctx is not None:
            ctx.enter_context(loop_ctx)
        alt_in = mode == "v4alt"
        ys_pair = [None]
        for _ in range(rep):
            for s in range(nsup):
                xs = xp.tile([128, 512 * nbands], F32R)
                in_eng = (nc.scalar if (alt_in and s % 2) else nc.sync)
                in_eng.dma_start(as3d(xs[:]), xv[s])
                if out_pair:
                    if s % 2 == 0:
                        ysb = yp.tile([128, 1024 * nbands], F32)
                        ys_pair[0] = ysb
                    ys = ys_pair[0][:, (s % 2) * 1024:(s % 2 + 1) * 1024]
                else:
                    ysb = yp.tile([128, 512 * nbands], F32)
                    ys = ysb[:]
                for g in range(nbands // 2):
                    halves(xs[:, g * 1024:(g + 1) * 1024], tp,
                           ys[:, g * 1024:(g + 1) * 1024])
                # SWDGE out-DMA: casts f32->bf16 in-flight when bf16_out
                if out_pair:
                    if s % 2 == 1:
                        nc.gpsimd.dma_start(
                            yv4[s // 2],
                            ys_pair[0][:].rearrange("p (t w) -> p t w",
                                                    t=2 * nbands))
                else:
                    nc.gpsimd.dma_start(yv[s], as3d(ys))

    nc.compile()
    return nc


def _get_nc(mode=MODE, rep=1, hwloop=0):
    key = (mode, rep, hwloop)
    if key not in _NC_CACHE:
        _NC_CACHE[key] = _build_nc(mode=mode, rep=rep, hwloop=hwloop)
    return _NC_CACHE[key]


def run_sharded(x, dct_basis, mode=MODE):
    """Shard batch over 8 cores, run the Bass kernel SPMD, gather output."""
    from concourse import bass_utils

    x = np.ascontiguousarray(np.asarray(x), dtype=np.float32)
    dct_basis = np.asarray(dct_basis, dtype=np.float32)
    assert x.shape == (B, C, H, W), x.shape

    bt = np.ascontiguousarray(
        np.kron(np.eye(16, dtype=np.float32), dct_basis).T.astype(np.float32)
    )
    bpc = B // N_CORES
    in_maps = []
    for c in range(N_CORES):
        m = {"x": x[c * bpc:(c + 1) * bpc].reshape(ROWS_PER_CORE, 512),
             "bt": bt}
        if mode == "v4c":
            m["bt2"] = bt
        in_maps.append(m)
    nc = _get_nc(mode=mode)
    res = bass_utils.run_bass_kernel_spmd(nc, in_maps, list(range(N_CORES)))
    out = np.concatenate(
        [np.asarray(res.results[c]["y"]).astype(np.float32).reshape(
            bpc, C, H, W) for c in range(N_CORES)],
        axis=0,
    )
    return out


def kernel(x, dct_basis):
    return run_sharded(x, dct_basis, mode=MODE)
